# revision 2
# baseline (speedup 1.0000x reference)
"""Trainium2 Bass kernel for BoundingBoxRegressorAndMultiLabelClassifier.

Data-parallel over batch: each of 8 cores processes 2 samples end-to-end.
ROI align is reformulated as a single joint matmul per sample:
    pooled[c, (n,p,q)] = sum_{h,w} feat[c,h,w] * Qy[n,p,h] * Qx[n,q,w]
with the interpolation matrix M^T[(h,w),(n,p,q)] built on the host from the
box coords (pure index math), so the heavy gather/arith stays on device as
dense matmuls. The roi_fc layer consumes pooled directly via stride-49 views
(no transposes). Heads run on PE from a transposed feats, with mask+reduce
diagonal extraction on DVE.

Matmuls use float32r (fp32 with 11-bit mantissa, full PE rate at N>=256).
"""
import numpy as np
import concourse.bass as bass
import concourse.tile as tile
from concourse import bacc, mybir
from concourse import bass_utils

# ---- problem constants (hardcoded per contract) ----
B, C, H = 16, 512, 32
HID, NB, S, P, SR = 512, 36, 26, 7, 2
SCALE = 32.0
N_LOC, N_LOC_LAB = 12, 8
N_GRP, GRP_SZ, N_GRP_LAB = 4, 6, 16

NCORES = 8
SPC = B // NCORES          # samples per core = 2
PQ = P * P                 # 49
NPQ = S * PQ               # 1274
CCH = C // 128             # 4 channel chunks
KCH = (H * H) // 128       # 8 hw chunks
MROWS = SPC * S            # 52 box-rows per core
NCHUNKS = [(0, 512), (512, 512), (1024, NPQ - 1024)]

f32 = mybir.dt.float32
f32r = mybir.dt.float32r


def _r(x: np.ndarray) -> np.ndarray:
    """RNE-round fp32 to fp32r (11-bit mantissa) so host data matches what the
    PE consumes; keeps sim == hw."""
    b = np.ascontiguousarray(x, dtype=np.float32).view(np.uint32)
    r = b + np.uint32(0x7FF) + ((b >> np.uint32(12)) & np.uint32(1))
    return (r & np.uint32(0xFFFFF000)).view(np.float32)


def _interp_mats(coords: np.ndarray):
    """coords (B,S,4) -> Qy, Qx (B,S,P,H) fp32, pooling+validity folded in."""
    c = coords.astype(np.float32)
    x1 = c[..., 0] * np.float32(SCALE)
    y1 = c[..., 1] * np.float32(SCALE)
    x2 = c[..., 2] * np.float32(SCALE)
    y2 = c[..., 3] * np.float32(SCALE)
    rw = np.maximum(x2 - x1, np.float32(1.0))
    rh = np.maximum(y2 - y1, np.float32(1.0))
    bw = rw / np.float32(P)
    bh = rh / np.float32(P)
    off = ((np.arange(P, dtype=np.float32)[:, None]
            + (np.arange(SR, dtype=np.float32) + np.float32(0.5)) / np.float32(SR))
           .reshape(-1))                                   # (P*SR,)
    ys = y1[..., None] + off * bh[..., None]               # (B,S,14)
    xs = x1[..., None] + off * bw[..., None]

    eye = np.eye(H, dtype=np.float32)

    def qmat(t):
        valid = ((t > -1.0) & (t < H)).astype(np.float32)
        tc = np.clip(t, np.float32(0.0), np.float32(H - 1))
        lo = np.floor(tc).astype(np.int64)
        hi = np.minimum(lo + 1, H - 1)
        fr = (tc - lo.astype(np.float32)).astype(np.float32)
        R = (eye[lo] * ((1.0 - fr) * valid)[..., None]
             + eye[hi] * (fr * valid)[..., None])          # (B,S,14,H)
        return R.reshape(B, S, P, SR, H).mean(axis=3)      # (B,S,P,H)

    return qmat(ys), qmat(xs)


_BUILT = None


def _build_program():
    """Build + compile the Bass program once per process."""
    nc = bacc.Bacc("TRN2", target_bir_lowering=False, debug=False,
                   enable_asserts=False, num_devices=NCORES)

    d = {}
    def din(name, shape, dt=f32r):
        d[name] = nc.dram_tensor(name, list(shape), dt, kind="ExternalInput").ap()
    def dout(name, shape, dt=f32):
        d[name] = nc.dram_tensor(name, list(shape), dt, kind="ExternalOutput").ap()

    din("featT", (SPC, H * H, C))
    din("MT", (SPC, H * H, NPQ))
    din("Wst", (PQ, 128, CCH * HID))
    din("fcbias", (1, HID))
    din("ones", (1, 128))
    din("ident", (128, 128), f32)
    din("cwT", (HID, S * 4))
    din("pwT", (HID, S))
    din("lwT", (HID, N_LOC * N_LOC_LAB))
    din("gwT", (GRP_SZ, HID, N_GRP * N_GRP_LAB))
    din("maskC", (S * 4, MROWS), f32)
    din("maskP", (S, MROWS), f32)
    din("maskL", (N_LOC * N_LOC_LAB, MROWS), f32)
    din("maskG", (N_GRP * N_GRP_LAB, GRP_SZ * SPC * N_GRP), f32)
    din("cbias", (S * 4, 1), f32)
    din("pbias", (S, 1), f32)
    din("lbias", (N_LOC * N_LOC_LAB, 1), f32)
    din("gbias", (N_GRP * N_GRP_LAB, 1), f32)
    din("predT", (S * 4, SPC), f32)
    din("whT", (S * 4, SPC), f32)
    dout("o_ref", (S * 4, SPC))
    dout("o_pres", (S, SPC))
    dout("o_loc", (N_LOC * N_LOC_LAB, SPC))
    dout("o_grp", (N_GRP * N_GRP_LAB, SPC))

    NLOCR = N_LOC * N_LOC_LAB   # 96
    NGRPR = N_GRP * N_GRP_LAB   # 64

    with tile.TileContext(nc) as tc:
        with (tc.tile_pool(name="const", bufs=1) as cp,
              tc.tile_pool(name="big", bufs=1) as bp,
              tc.tile_pool(name="wst", bufs=3) as wp,
              tc.tile_pool(name="wk", bufs=2) as wk,
              tc.tile_pool(name="psI", bufs=3, space="PSUM") as psI,
              tc.tile_pool(name="psF", bufs=1, space="PSUM") as psF,
              tc.tile_pool(name="psT", bufs=1, space="PSUM") as psT,
              tc.tile_pool(name="psH", bufs=2, space="PSUM") as psH):

            # ---- persistent loads ----
            featT = bp.tile([128, SPC * KCH * C], f32r, tag="featT")
            MT = bp.tile([128, SPC * KCH * NPQ], f32r, tag="MT")
            for s in range(SPC):
                for k in range(KCH):
                    nc.sync.dma_start(
                        featT[:, (s * KCH + k) * C:(s * KCH + k + 1) * C],
                        d["featT"][s, k * 128:(k + 1) * 128, :])
                    nc.sync.dma_start(
                        MT[:, (s * KCH + k) * NPQ:(s * KCH + k + 1) * NPQ],
                        d["MT"][s, k * 128:(k + 1) * 128, :])

            def cload(name, shape, dt=f32r):
                t = cp.tile(list(shape), dt, tag=name)
                nc.sync.dma_start(t[:], d[name][:])
                return t
            ones_sb = cload("ones", (1, 128))
            fcb_sb = cload("fcbias", (1, HID))
            id_sb = cload("ident", (128, 128), f32)
            maskC = cload("maskC", (S * 4, MROWS), f32)
            maskP = cload("maskP", (S, MROWS), f32)
            maskL = cload("maskL", (NLOCR, MROWS), f32)
            maskG = cload("maskG", (NGRPR, GRP_SZ * SPC * N_GRP), f32)
            cbias = cload("cbias", (S * 4, 1), f32)
            pbias = cload("pbias", (S, 1), f32)
            lbias = cload("lbias", (NLOCR, 1), f32)
            gbias = cload("gbias", (NGRPR, 1), f32)
            predT = cload("predT", (S * 4, SPC), f32)
            whT = cload("whT", (S * 4, SPC), f32)

            cwT = cp.tile([128, CCH * S * 4], f32r, tag="cwT")
            pwT = cp.tile([128, CCH * S], f32r, tag="pwT")
            lwT = cp.tile([128, CCH * NLOCR], f32r, tag="lwT")
            gwT = cp.tile([128, GRP_SZ * CCH * NGRPR], f32r, tag="gwT")
            for hc in range(CCH):
                nc.sync.dma_start(cwT[:, hc * S * 4:(hc + 1) * S * 4],
                                  d["cwT"][hc * 128:(hc + 1) * 128, :])
                nc.sync.dma_start(pwT[:, hc * S:(hc + 1) * S],
                                  d["pwT"][hc * 128:(hc + 1) * 128, :])
                nc.sync.dma_start(lwT[:, hc * NLOCR:(hc + 1) * NLOCR],
                                  d["lwT"][hc * 128:(hc + 1) * 128, :])
                for m in range(GRP_SZ):
                    nc.sync.dma_start(
                        gwT[:, (m * CCH + hc) * NGRPR:(m * CCH + hc + 1) * NGRPR],
                        d["gwT"][m, hc * 128:(hc + 1) * 128, :])

            # ---- ROI-align interp as joint matmuls ----
            pooled = [bp.tile([128, SPC * NPQ], f32r, tag=f"pooled{cc}",
                              name=f"pooled{cc}")
                      for cc in range(CCH)]
            for s in range(SPC):
                for cc in range(CCH):
                    for (n0, nn) in NCHUNKS:
                        pt = psI.tile([128, nn], f32, tag="psI")
                        for k in range(KCH):
                            base = (s * KCH + k)
                            nc.tensor.matmul(
                                pt[:],
                                featT[:, base * C + cc * 128: base * C + cc * 128 + 128],
                                MT[:, base * NPQ + n0: base * NPQ + n0 + nn],
                                start=(k == 0), stop=(k == KCH - 1))
                        nc.vector.tensor_copy(
                            pooled[cc][:, s * NPQ + n0: s * NPQ + n0 + nn], pt[:])

            # ---- roi_fc: feats[(s,n), j] accumulated over 196 K-chunks ----
            fps = psF.tile([MROWS, HID], f32, tag="psF")
            nc.tensor.matmul(fps[:], ones_sb[:, :MROWS], fcb_sb[:],
                             start=True, stop=False)
            for pq in range(PQ):
                wt = wp.tile([128, CCH * HID], f32r, tag="wst")
                nc.sync.dma_start(wt[:], d["Wst"][pq])
                for cc in range(CCH):
                    nc.tensor.matmul(
                        fps[:],
                        pooled[cc][:, pq::PQ],
                        wt[:, cc * HID:(cc + 1) * HID],
                        start=False, stop=(pq == PQ - 1 and cc == CCH - 1))
            feats = wk.tile([MROWS, HID], f32, tag="feats")
            nc.vector.tensor_relu(feats[:], fps[:])

            # ---- transpose feats -> featsT (h-part, (s,n)-free) ----
            featsT = wk.tile([128, CCH * MROWS], f32r, tag="featsT")
            for hc in range(CCH):
                tp = psT.tile([128, MROWS], f32, tag="psT")
                nc.tensor.transpose(tp[:], feats[:, hc * 128:(hc + 1) * 128],
                                    id_sb[:MROWS, :MROWS])
                nc.vector.tensor_copy(featsT[:, hc * MROWS:(hc + 1) * MROWS], tp[:])

            # ---- heads (matmul + mask + segment-reduce) ----
            def head_mm(wtile, nrows, wstride):
                hp = psH.tile([nrows, MROWS], f32, tag="psH")
                for hc in range(CCH):
                    nc.tensor.matmul(
                        hp[:], wtile[:, hc * wstride: hc * wstride + nrows],
                        featsT[:, hc * MROWS:(hc + 1) * MROWS],
                        start=(hc == 0), stop=(hc == CCH - 1))
                return hp

            def mask_reduce(hp, nrows, mask, tagn):
                mskd = wk.tile([nrows, MROWS], f32, tag="m" + tagn)
                nc.vector.tensor_mul(mskd[:], hp[:], mask[:])
                red = wk.tile([nrows, SPC], f32, tag="r" + tagn)
                nc.vector.reduce_sum(
                    red[:], mskd[:].rearrange("p (s n) -> p s n", s=SPC),
                    axis=mybir.AxisListType.X)
                return red

            # coords head -> refined
            hpC = head_mm(cwT, S * 4, S * 4)
            redC = mask_reduce(hpC, S * 4, maskC, "C")
            nc.vector.tensor_scalar_add(redC[:], redC[:], cbias[:])
            nc.vector.tensor_mul(redC[:], redC[:], whT[:])
            nc.vector.tensor_add(redC[:], redC[:], predT[:])
            nc.sync.dma_start(d["o_ref"][:], redC[:])

            # presence head
            hpP = head_mm(pwT, S, S)
            redP = mask_reduce(hpP, S, maskP, "P")
            nc.vector.tensor_scalar_add(redP[:], redP[:], pbias[:])
            nc.sync.dma_start(d["o_pres"][:], redP[:])

            # loc head
            hpL = head_mm(lwT, NLOCR, NLOCR)
            redL = mask_reduce(hpL, NLOCR, maskL, "L")
            nc.vector.tensor_scalar_add(redL[:], redL[:], lbias[:])
            nc.sync.dma_start(d["o_loc"][:], redL[:])

            # grp head: per-member matmuls, columns (s, g')
            gp = psH.tile([NGRPR, GRP_SZ * SPC * N_GRP], f32, tag="psH")
            for m in range(GRP_SZ):
                for hc in range(CCH):
                    rhsv = (featsT[:, hc * MROWS:(hc + 1) * MROWS]
                            .rearrange("p (s n) -> p s n", s=SPC)
                            [:, :, m:m + (N_GRP - 1) * GRP_SZ + 1:GRP_SZ])
                    nc.tensor.matmul(
                        gp[:, m * SPC * N_GRP:(m + 1) * SPC * N_GRP],
                        gwT[:, (m * CCH + hc) * NGRPR:(m * CCH + hc + 1) * NGRPR],
                        rhsv,
                        start=(hc == 0), stop=(hc == CCH - 1))
            mskdG = wk.tile([NGRPR, GRP_SZ * SPC * N_GRP], f32, tag="mG")
            nc.vector.tensor_mul(mskdG[:], gp[:], maskG[:])
            r1 = wk.tile([NGRPR, GRP_SZ * SPC], f32, tag="r1G")
            nc.vector.reduce_sum(
                r1[:], mskdG[:].rearrange("p (m s g) -> p m s g", s=SPC, g=N_GRP),
                axis=mybir.AxisListType.X)
            redG = wk.tile([NGRPR, SPC], f32, tag="rG")
            nc.vector.reduce_sum(
                redG[:], r1[:].rearrange("p (m s) -> p s m", s=SPC),
                axis=mybir.AxisListType.X)
            nc.vector.tensor_scalar_add(redG[:], redG[:], gbias[:])
            nc.sync.dma_start(d["o_grp"][:], redG[:])

    nc.compile()
    return nc


def kernel(**inputs) -> tuple:
    global _BUILT
    if _BUILT is None:
        _BUILT = _build_program()
    nc = _BUILT

    lf = np.asarray(inputs["local_features"], np.float32)
    coords = np.asarray(inputs["pred_bbox_coords"], np.float32)
    Wfc = np.asarray(inputs["roi_fc_W"], np.float32)
    fcb = np.asarray(inputs["roi_fc_b"], np.float32)
    coords_W = np.asarray(inputs["coords_W"], np.float32)
    coords_b = np.asarray(inputs["coords_b"], np.float32)
    pres_W = np.asarray(inputs["pres_W"], np.float32)
    pres_b = np.asarray(inputs["pres_b"], np.float32)
    loc_W = np.asarray(inputs["loc_W"], np.float32)
    loc_b = np.asarray(inputs["loc_b"], np.float32)
    grp_W = np.asarray(inputs["grp_W"], np.float32)
    grp_b = np.asarray(inputs["grp_b"], np.float32)
    loc_idx = np.asarray(inputs["loc_idx"], np.int64)
    grp_idx = np.asarray(inputs["grp_idx"], np.int64)

    # ---- host prep ----
    Qy, Qx = _interp_mats(coords)
    # MT[b] (1024, NPQ), columns (n,p,q) n-major
    MT = np.einsum("bnph,bnqw->bhwnpq", Qy, Qx).reshape(B, H * H, NPQ)
    featT = lf.transpose(0, 2, 3, 1).reshape(B, H * H, C)

    Wr = Wfc.reshape(HID, C, PQ)
    Wst = (Wr.transpose(2, 1, 0).reshape(PQ, CCH, 128, HID)
           .transpose(0, 2, 1, 3).reshape(PQ, 128, CCH * HID))

    cwT = coords_W.transpose(2, 0, 1).reshape(HID, S * 4)
    pwT = pres_W.T
    lwT = loc_W[...].transpose(2, 0, 1).reshape(HID, N_LOC * N_LOC_LAB)
    gw4 = grp_W.reshape(N_GRP, N_GRP_LAB, GRP_SZ, HID)
    gwT = np.stack([gw4[:, :, m, :].transpose(2, 0, 1)
                    .reshape(HID, N_GRP * N_GRP_LAB) for m in range(GRP_SZ)])

    # masks
    maskC = np.zeros((S * 4, MROWS), np.float32)
    for n in range(S):
        for o in range(4):
            for s in range(SPC):
                maskC[n * 4 + o, s * S + n] = 1.0
    maskP = np.zeros((S, MROWS), np.float32)
    for n in range(S):
        for s in range(SPC):
            maskP[n, s * S + n] = 1.0
    maskL = np.zeros((N_LOC * N_LOC_LAB, MROWS), np.float32)
    for l in range(N_LOC):
        for o in range(N_LOC_LAB):
            for s in range(SPC):
                maskL[l * N_LOC_LAB + o, s * S + int(loc_idx[l])] = 1.0
    # grp member matmul columns are (s, g'); check grp_idx matches the strided view
    exp_gidx = np.arange(N_GRP * GRP_SZ).reshape(N_GRP, GRP_SZ)
    assert np.array_equal(grp_idx, exp_gidx), "grp_idx pattern unsupported"
    assert np.array_equal(loc_idx, np.arange(N_LOC)) or True
    maskG = np.zeros((N_GRP * N_GRP_LAB, GRP_SZ * SPC * N_GRP), np.float32)
    for g in range(N_GRP):
        for o in range(N_GRP_LAB):
            for m in range(GRP_SZ):
                for s in range(SPC):
                    maskG[g * N_GRP_LAB + o, m * SPC * N_GRP + s * N_GRP + g] = 1.0

    cbias = coords_b.reshape(S * 4, 1).astype(np.float32)
    pbias = pres_b.reshape(S, 1).astype(np.float32)
    lbias = loc_b.reshape(N_LOC * N_LOC_LAB, 1).astype(np.float32)
    gbias = grp_b.reshape(N_GRP * N_GRP_LAB, 1).astype(np.float32)

    shared = {
        "Wst": _r(Wst), "fcbias": _r(fcb.reshape(1, HID)),
        "ones": _r(np.ones((1, 128), np.float32)),
        "ident": np.eye(128, dtype=np.float32),
        "cwT": _r(cwT), "pwT": _r(pwT), "lwT": _r(lwT), "gwT": _r(gwT),
        "maskC": maskC, "maskP": maskP, "maskL": maskL, "maskG": maskG,
        "cbias": cbias, "pbias": pbias, "lbias": lbias, "gbias": gbias,
    }

    w = coords[..., 2] - coords[..., 0]
    h = coords[..., 3] - coords[..., 1]
    wh = np.stack([w, h, w, h], axis=-1)  # (B,S,4)

    in_maps = []
    for k in range(NCORES):
        sl = slice(k * SPC, (k + 1) * SPC)
        m = dict(shared)
        m["featT"] = _r(featT[sl])
        m["MT"] = _r(MT[sl])
        m["predT"] = np.ascontiguousarray(
            coords[sl].transpose(1, 2, 0).reshape(S * 4, SPC))
        m["whT"] = np.ascontiguousarray(
            wh[sl].transpose(1, 2, 0).reshape(S * 4, SPC))
        in_maps.append(m)

    res = bass_utils.run_bass_kernel_spmd(nc, in_maps, core_ids=list(range(NCORES)))
    kernel.last_result = res

    refined = np.zeros((B, S, 4), np.float32)
    presence = np.zeros((B, S), np.float32)
    mlc = np.zeros((B, N_LOC * N_LOC_LAB + N_GRP * N_GRP_LAB), np.float32)
    for k in range(NCORES):
        r = res.results[k]
        for s in range(SPC):
            b = k * SPC + s
            refined[b] = r["o_ref"][:, s].reshape(S, 4)
            presence[b] = r["o_pres"][:, s]
            mlc[b, :N_LOC * N_LOC_LAB] = r["o_loc"][:, s]
            mlc[b, N_LOC * N_LOC_LAB:] = r["o_grp"][:, s]
    return refined, presence, mlc
